# revision 31
# baseline (speedup 1.0000x reference)
"""CRF loss kernel for Trainium2 (8 NeuronCores, SPMD data-parallel over batch).

V7 design — segmented rank-1 stitching (V5 core), startup/tail tuned:
  The T=512-step forward algorithm is split into S=16 time segments; per
  segment a forward chain (init folded host-side) and a backward chain run
  31 lockstep matmul+multiply rounds in 2 groups, after which the segment
  transfer operators are numerically rank-1 and lnZ telescopes into
  per-segment dot products (see V5 notes).  The scan round is at the DVE
  floor, so V7 keeps V5's scan + DMA-queue scheme exactly (restructuring Q
  onto a single HW queue in V6 slowed every engine ~18%) and tunes the ends:
  - slot-0 of Q is PRESCALED host-side (start/end/wbar factors folded in,
    bf16 for range: TRN e4m3 is NaN above 240) so round 1's matmul reads
    the DMA'd tile directly — no init ops or sc0/scm on the critical path.
  - stitch: dprod moves to the ACT engine (per-partition scale), the 29
    per-pair partition-reduce matmuls become 15 two-pair-packed matmuls,
    and fold+transpose fuse into one matmul (lhsT = per-partition logZ
    column, rhs = [I64;I64]).
  - output DMA on the scalar HW queue.
  Q = exp(emis - SHIFT) is fp8-e4m3 host-side (values < 1; safe); the
  numerator is computed host-side in f64.  ln of the unbounded dot products
  is exponent/mantissa split (the Ln table overflows above ~1e16).
"""

import os
import sys

import numpy as np
import ml_dtypes

for _p in ("/opt/trn_rl_repo", "/opt/pypackages"):
    if os.path.isdir(_p) and _p not in sys.path:
        sys.path.append(_p)

import concourse.bass as bass
import concourse.bacc as bacc
import concourse.mybir as mybir
import concourse.tile as tile
from concourse.alu_op_type import AluOpType
from contextlib import ExitStack

B, T, C = 512, 512, 64
NCORES = 8
BLOC = B // NCORES            # 64
SHIFT = 4.65
S = 16                        # time segments
L = T // S                    # 32 steps per segment
R = L - 1                     # matmul+mult rounds per pair
P = S - 1                     # pair-chain tiles
WGRP = P * BLOC // 2          # 480: equal group widths (the round chain is
                              # set by the wider group, so split pair 7's 64
                              # batch columns across the two groups)
WS0 = 8 * BLOC                # slot-0 DMA padded to 512B rows (480B rows
                              # cost ~1.5µs extra completion lag — measured)
CB = [1, 2, 4, 8, 16, 32]     # Q chunk slot boundaries (slot 0 rides its own
                              # bf16 tensors); finer early chunks so rounds
                              # 1-7 aren't gated on late bulk arrivals
# queue split (scalar HW + gpsimd SW) is byte-balanced and hardcoded in the
# DMA issue sequence below

AF = mybir.ActivationFunctionType
bf16 = ml_dtypes.bfloat16
fp8 = ml_dtypes.float8_e4m3


def build_crf_program():
    dt = mybir.dt
    f32, b16, u32, f8 = dt.float32, dt.bfloat16, dt.uint32, dt.float8e4
    NCHUNK = len(CB) - 1
    assert CB[-1] == R + 1
    G = 2
    LN2 = float(np.log(2.0))

    nc = bacc.Bacc("TRN2", target_bir_lowering=False, debug=False,
                   num_devices=NCORES)
    wg = [WGRP, WGRP]
    qs0_d = [nc.dram_tensor(f"q{g}s0", [2 * C, WS0], b16,
                            kind="ExternalInput").ap() for g in range(G)]
    qd = [nc.dram_tensor(f"q{g}", [2 * C, R * wg[g]], f8,
                         kind="ExternalInput").ap() for g in range(G)]
    wpair_d = nc.dram_tensor("wpair", [2 * C, 2 * C], b16, kind="ExternalInput").ap()
    wzt_d = nc.dram_tensor("wzt", [C, C], b16, kind="ExternalInput").ap()
    scw_d = nc.dram_tensor("scw", [2 * C, 1], f32, kind="ExternalInput").ap()
    foldI_d = nc.dram_tensor("foldI", [2 * C, C], f32, kind="ExternalInput").ap()
    out_logZ = nc.dram_tensor("out_logZ", [1, BLOC], f32, kind="ExternalOutput").ap()

    with ExitStack() as ctx:
        tc = ctx.enter_context(tile.TileContext(nc))
        const = ctx.enter_context(tc.tile_pool(name="const", bufs=1))
        qpool = ctx.enter_context(tc.tile_pool(name="q", bufs=1))
        stp = [ctx.enter_context(tc.tile_pool(name=f"st{g}", bufs=2))
               for g in range(G)]
        misc = ctx.enter_context(tc.tile_pool(name="misc", bufs=1))
        psp = [ctx.enter_context(tc.tile_pool(name=f"ps{g}", bufs=1, space="PSUM"))
               for g in range(G)]
        psz = ctx.enter_context(tc.tile_pool(name="psz", bufs=1, space="PSUM"))
        psr = ctx.enter_context(tc.tile_pool(name="psr", bufs=1, space="PSUM"))

        # ---- tile allocation (order fixed: SBUF layout is perf-sensitive;
        # matches the measured-best V7.1 layout exactly) ----
        wpair = const.tile([2 * C, 2 * C], b16)
        qs0 = [const.tile([2 * C, WS0], b16, tag=f"q{g}s0", name=f"q{g}s0")
               for g in range(G)]
        wzt = const.tile([2 * C, C], b16)
        scw = const.tile([2 * C, 1], f32)
        foldI = const.tile([2 * C, C], f32)
        ones128 = const.tile([2 * C, 1], b16)
        nc.vector.memset(ones128[:], 1.0)
        qt = [[None] * NCHUNK for _ in range(G)]
        for c in range(NCHUNK):
            for g in range(G):
                nsl = CB[c + 1] - CB[c]
                qt[g][c] = qpool.tile([2 * C, nsl * wg[g]], f8,
                                      tag=f"q{g}c{c}", name=f"q{g}c{c}")

        # ---- DMA issue order: each engine's DMA_DIRECT2D descriptors go out
        # serially (~600ns each), so round-gating transfers (wpair, slot-0,
        # chunk 0/1) come first on both queues; stitch-only constants
        # (scw/wzt/foldI) issue after the early chunks ----
        def q_dma(g, c, eng):
            w = wg[g]
            eng.dma_start(qt[g][c][:],
                          qd[g][:, (CB[c] - 1) * w:(CB[c + 1] - 1) * w])

        # slot-0 of g0 gates the very first matmul: split it by partition
        # halves across both queues so it lands ~0.7us earlier
        nc.scalar.dma_start(qs0[0][0:C, :], qs0_d[0][0:C, :])
        nc.gpsimd.dma_start(qs0[0][C:2 * C, :], qs0_d[0][C:2 * C, :])
        nc.scalar.dma_start(wpair[:], wpair_d)
        nc.gpsimd.dma_start(qs0[1][:], qs0_d[1])
        q_dma(0, 0, nc.scalar)
        q_dma(1, 0, nc.gpsimd)
        q_dma(1, 1, nc.scalar)
        q_dma(0, 1, nc.gpsimd)
        nc.scalar.dma_start(scw[:], scw_d)
        nc.gpsimd.dma_start(wzt[C:2 * C, :], wzt_d)
        nc.gpsimd.dma_start(foldI[:], foldI_d)
        q_dma(0, 2, nc.scalar)
        q_dma(1, 2, nc.gpsimd)
        q_dma(1, 3, nc.scalar)
        q_dma(0, 3, nc.gpsimd)
        q_dma(0, 4, nc.scalar)
        q_dma(1, 4, nc.gpsimd)

        def q_slice(g, r):
            c = next(i for i in range(NCHUNK) if CB[i] <= r < CB[i + 1])
            w = wg[g]
            o = r - CB[c]
            return qt[g][c][:, o * w:(o + 1) * w]

        # ---- scan: R rounds x (matmul + multiply) per group ----
        # round-1 matmul reads the prescaled slot-0 (first 480 of the padded
        # 512-col transfer)
        st = [qs0[g][:, 0:WGRP] for g in range(G)]
        for r in range(1, R + 1):
            for g in range(G):
                ps = psp[g].tile([2 * C, wg[g]], f32, tag=f"s{g}", name=f"s{g}")
                nc.tensor.matmul(ps[:], lhsT=wpair[:], rhs=st[g][:],
                                 start=True, stop=True)
                sn = stp[g].tile([2 * C, wg[g]], b16, tag=f"st{g}",
                                 name=f"sn{g}")
                nc.vector.tensor_tensor(sn[:], ps[:], q_slice(g, r),
                                        op=AluOpType.mult)
                st[g] = sn

        # ---- stitch ----
        # z = E y on partitions 0:64 per pair
        pz = []
        for g in range(G):
            z = psz.tile([C, wg[g]], f32, tag=f"z{g}", name=f"z{g}")
            nc.tensor.matmul(z[:], lhsT=wzt[C:2 * C, :],
                             rhs=st[g][C:2 * C, :], start=True, stop=True)
            pz.append(z)

        # D products w_bar*y_p (pairs 1..14) on the ACT engine (per-partition
        # scale), freeing the DVE for the N products.  Canonical bottom cols
        # [64,960) -> dprod cols [0,896), group boundary at canonical 480.
        dprod = misc.tile([2 * C, (P - 1) * BLOC], b16, tag="dprod")
        nc.scalar.mul(dprod[C:2 * C, 0:WGRP - BLOC],
                      st[0][C:2 * C, BLOC:WGRP], scw[C:2 * C, :1])
        nc.scalar.mul(dprod[C:2 * C, WGRP - BLOC:],
                      st[1][C:2 * C, :], scw[C:2 * C, :1])

        # two-pair-packed partition reduces: [64,128]^T @ ones -> [128,1]
        # (partitions 0:64 = even pair's batches, 64:128 = odd pair's).
        # d-MMs are emitted FIRST: the PE stream executes in program order,
        # and dprod is ready while the DVE is still producing nprod.
        ncol2 = psr.tile([2 * C, 8], f32, tag="ncol2")
        dcol2 = psr.tile([2 * C, 7], f32, tag="dcol2")
        nc.vector.memset(ncol2[C:2 * C, 7:8], 1.0)       # pad: ln contrib 0
        for k in range(7):
            nc.tensor.matmul(dcol2[:, k:k + 1],
                             lhsT=dprod[C:2 * C, k * 2 * BLOC:(k + 1) * 2 * BLOC],
                             rhs=ones128[C:2 * C, :], start=True, stop=True)

        # N products z_p * x_{p-1 mod P}: batched TTs over contiguous
        # canonical runs, spliced at the group boundary (z stays in PSUM: an
        # ACT bf16 copy enables DVE 2x here but its serial latency costs
        # more than 2x saves — measured).  Order matches the n-MM blocks.
        nprod = misc.tile([C, P * BLOC], b16, tag="nprod")

        def emit_nprod(nc0, nc1, xc0):
            """nprod canonical [nc0,nc1) = z[nc0,nc1) * x[xc0,+len)."""
            offs = {0, nc1 - nc0}
            for base in (nc0, xc0):
                if base < WGRP < base + (nc1 - nc0):
                    offs.add(WGRP - base)
            offs = sorted(offs)
            for a, b in zip(offs, offs[1:]):
                zg, zl = (nc0 + a) // WGRP, (nc0 + a) % WGRP
                xg, xl = (xc0 + a) // WGRP, (xc0 + a) % WGRP
                n = b - a
                nc.vector.tensor_tensor(nprod[:, nc0 + a:nc0 + b],
                                        pz[zg][0:C, zl:zl + n],
                                        st[xg][0:C, xl:xl + n],
                                        op=AluOpType.mult)

        emit_nprod(BLOC, 8 * BLOC, 0)                    # pairs 1..7
        emit_nprod(0, BLOC, 14 * BLOC)                   # pair 0 (x: pair 14)
        emit_nprod(8 * BLOC, 9 * BLOC, 7 * BLOC)         # pair 8 (x: pair 7)
        emit_nprod(9 * BLOC, 15 * BLOC, 8 * BLOC)        # pairs 9..14

        for k in (1, 2, 3, 0, 4, 5, 6):                  # blocks as TTs land
            nc.tensor.matmul(ncol2[:, k:k + 1],
                             lhsT=nprod[0:C, k * 2 * BLOC:(k + 1) * 2 * BLOC],
                             rhs=ones128[0:C, :], start=True, stop=True)
        nc.tensor.matmul(ncol2[0:C, 7:8],                # pair 14 alone
                         lhsT=nprod[0:C, 14 * BLOC:15 * BLOC],
                         rhs=ones128[0:C, :], start=True, stop=True)

        def ln_col(src_psum, n, tagp):
            """per-batch raw ln-sum (incl +127*ln2 per col bias) of positive
            f32 PSUM [128,n] of unbounded magnitude -> [128,1] f32."""
            sb = misc.tile([2 * C, n], f32, tag=f"{tagp}sb", name=f"{tagp}sb")
            nc.vector.tensor_copy(sb[:], src_psum)
            eb = misc.tile([2 * C, n], u32, tag=f"{tagp}eb", name=f"{tagp}eb")
            nc.vector.tensor_scalar(eb[:], sb[:].bitcast(u32), 23, None,
                                    op0=AluOpType.logical_shift_right)
            mant = misc.tile([2 * C, n], u32, tag=f"{tagp}mt", name=f"{tagp}mt")
            nc.vector.tensor_scalar(mant[:], sb[:].bitcast(u32),
                                    0x007FFFFF, 0x3F800000,
                                    op0=AluOpType.bitwise_and,
                                    op1=AluOpType.bitwise_or)
            lnm = misc.tile([2 * C, n], f32, tag=f"{tagp}lm", name=f"{tagp}lm")
            nc.scalar.activation(lnm[:], mant[:].bitcast(f32), AF.Ln)
            ls = misc.tile([2 * C, 1], f32, tag=f"{tagp}ls", name=f"{tagp}ls")
            nc.vector.tensor_reduce(ls[:], lnm[:], mybir.AxisListType.X,
                                    AluOpType.add)
            es = misc.tile([2 * C, 1], f32, tag=f"{tagp}es", name=f"{tagp}es")
            nc.vector.tensor_reduce(es[:], eb[:], mybir.AxisListType.X,
                                    AluOpType.add)
            out = misc.tile([2 * C, 1], f32, tag=f"{tagp}o", name=f"{tagp}o")
            nc.vector.scalar_tensor_tensor(out[:], es[:], LN2, ls[:],
                                           op0=AluOpType.mult,
                                           op1=AluOpType.add)
            return out

        ad = ln_col(dcol2[:], 7, "d")                    # d ready first
        an = ln_col(ncol2[:], 8, "n")
        logZcol = misc.tile([2 * C, 1], f32, tag="logZc")
        nc.vector.tensor_tensor(logZcol[:], an[:], ad[:],
                                op=AluOpType.subtract)
        # fused fold+transpose: out[0,b] = sum_j logZcol[j] * foldI[j,b]
        # with foldI = [I64; I64] -> top half + bottom half per batch
        zrow = psr.tile([1, BLOC], f32, tag="zrow")
        nc.tensor.matmul(zrow[:], lhsT=logZcol[:], rhs=foldI[:],
                         start=True, stop=True)
        zrow_sb = misc.tile([1, BLOC], f32, tag="zrsb")
        # bias: 15 numerator cols + 1 pad - 14 denominator cols => -254*ln2
        nc.vector.tensor_scalar(zrow_sb[:], zrow[:],
                                float(SHIFT * T - 254.0 * LN2), None,
                                op0=AluOpType.add)
        nc.scalar.dma_start(out_logZ, zrow_sb[:])

    nc.compile()
    return nc


_PROG_CACHE = {}


def _get_program():
    if "p" not in _PROG_CACHE:
        _PROG_CACHE["p"] = build_crf_program()
    return _PROG_CACHE["p"]


def host_prepare(emissions, tags, transitions, start_transitions,
                 end_transitions):
    """Per-core input maps + host (numerator) part."""
    em = np.asarray(emissions, np.float32)
    q = np.exp(em - np.float32(SHIFT)).astype(fp8)       # [B,T,C]
    E = np.exp(np.asarray(transitions, np.float64))
    wbar = E.sum(axis=0)                                  # (E^T 1)_j
    wpair = np.zeros((2 * C, 2 * C), np.float64)
    wpair[0:C, 0:C] = E
    wpair[C:2 * C, C:2 * C] = E.T
    wpair = wpair.astype(bf16)
    wzt = E.T.astype(bf16)                                # [64,64]
    scw = np.concatenate([np.ones(C), wbar]).astype(np.float32).reshape(2 * C, 1)
    foldI = np.concatenate([np.eye(C), np.eye(C)]).astype(np.float32)

    # slot-0 prescale factors: [exp(start); exp(end)] for pair 0,
    # [wbar; ones] for pairs >= 1 (the forward chain's first step folded)
    sc0 = np.concatenate([np.exp(np.asarray(start_transitions, np.float64)),
                          np.exp(np.asarray(end_transitions, np.float64))])
    scm = np.concatenate([wbar, np.ones(C)])

    # per-pair time maps (slot 0 = init, slots 1..R = rounds)
    tmap_top = np.empty((P, R + 1), np.int64)
    tmap_bot = np.empty((P, R + 1), np.int64)
    for p in range(P):
        t0, t1 = p * L, (p + 1) * L - 1
        if p == 0:
            tmap_top[0] = np.arange(0, R + 1)            # 0,1..R
            tmap_bot[0] = T - 1 - np.arange(0, R + 1)    # 511,510..
        else:
            tmap_top[p] = t0 + np.arange(0, R + 1)
            tmap_bot[p] = t1 - np.arange(0, R + 1)
    in_maps = []
    WG = P * BLOC // 2                                    # 480
    for cidx in range(NCORES):
        b0 = cidx * BLOC
        qc = q[b0:b0 + BLOC]                              # [64,512,64] fp8
        m = {"wpair": wpair, "wzt": wzt, "scw": scw, "foldI": foldI}
        # canonical pair-major layout, then slice at the group boundary
        big = np.empty((2 * C, R, P * BLOC), fp8)
        s0 = np.empty((2 * C, P * BLOC), np.float64)
        for p in range(P):
            cs = slice(p * BLOC, (p + 1) * BLOC)
            big[0:C, :, cs] = qc[:, tmap_top[p, 1:], :].transpose(2, 1, 0)
            big[C:2 * C, :, cs] = qc[:, tmap_bot[p, 1:], :].transpose(2, 1, 0)
            sc = sc0 if p == 0 else scm
            s0[0:C, cs] = (qc[:, tmap_top[p, 0], :].astype(np.float64).T
                           * sc[0:C, None])
            s0[C:2 * C, cs] = (qc[:, tmap_bot[p, 0], :].astype(np.float64).T
                               * sc[C:2 * C, None])
        for g in range(2):
            gs = slice(g * WG, (g + 1) * WG)
            m[f"q{g}"] = np.ascontiguousarray(
                big[:, :, gs].reshape(2 * C, R * WG))
            s0p = np.ones((2 * C, 8 * BLOC), np.float64)  # pad to 512B rows
            s0p[:, 0:WG] = s0[:, gs]
            m[f"q{g}s0"] = s0p.astype(bf16)
        in_maps.append(m)

    # host numerator (exact, f64)
    em64 = np.asarray(emissions, np.float64)
    tg = np.asarray(tags)
    st64 = np.asarray(start_transitions, np.float64)
    en64 = np.asarray(end_transitions, np.float64)
    tr64 = np.asarray(transitions, np.float64)
    num = (st64[tg[:, 0]]
           + np.take_along_axis(em64, tg[:, :, None], axis=2)[:, :, 0].sum(1)
           + tr64[tg[:, :-1], tg[:, 1:]].sum(1)
           + en64[tg[:, -1]])
    return in_maps, num


def kernel(emissions, tags, mask, transitions, start_transitions,
           end_transitions):
    from concourse.bass_utils import run_bass_kernel_spmd
    nc = _get_program()
    in_maps, num = host_prepare(emissions, tags, transitions,
                                start_transitions, end_transitions)
    res = run_bass_kernel_spmd(nc, in_maps, core_ids=list(range(NCORES)))
    vals = np.zeros(B, np.float64)
    for cidx in range(NCORES):
        b0 = cidx * BLOC
        logZ = res.results[cidx]["out_logZ"].reshape(BLOC).astype(np.float64)
        vals[b0:b0 + BLOC] = logZ - num[b0:b0 + BLOC]
    return np.float32(np.mean(vals))
